# revision 1
# baseline (speedup 1.0000x reference)
"""Trainium2 Bass kernel for nn_KKLayer (spectral channel-mix layer).

Math identity: the reference computes
    y = Re(IFFT2((A + iB) . conj(FFT2(x))))            (channel mix in freq domain)
Since channel mixing commutes with the spatial FFT and, for real x,
IFFT2(conj(FFT2(x))) is x spatially "negated" (h -> (-h) mod H, w -> (-w) mod W),
the whole layer collapses to
    y[b,o,h,w] = sum_i A[o,i] * x[b,i,(H-h)%H,(W-w)%W]
(betas drop out of the real part entirely).

Kernel: data-parallel over batch (8 batches -> 8 cores). Per core:
  - load alphas^T (stationary matmul weights) + x[b] into SBUF (8 x 1MB chunks)
  - 32 matmuls [K=128,M=128,N=512] -> PSUM
  - PSUM->SBUF copies apply the (h,w) flip via negative-stride APs
  - contiguous ~1MB DMA-out chunks

Single-wait discipline: TRN2 instructions carry at most ONE semaphore wait.
 - a 1-col "probe" matmul per x-chunk (both operands from the chunk) absorbs
   the chunk-DMA wait on PE; real matmuls then only wait on PSUM-slot reuse
 - all copies feeding one output chunk run on one engine, so each output DMA
   and each PSUM-slot reuse waits on a single engine
"""

import numpy as np

import concourse.bass as bass
import concourse.bacc as bacc
import concourse.mybir as mybir
from concourse import tile
from concourse.bass_utils import run_bass_kernel_spmd

B, CIN, COUT, H, W = 8, 128, 128, 128, 128
HW = H * W          # 16384
BLK = 512           # matmul free dim (one PSUM bank of fp32)
NBLK = HW // BLK    # 32 blocks; block j covers h rows 4j..4j+3
N_CORES = 8

F32 = mybir.dt.float32

# output chunks (offset by 1 row so no 4-row block straddles a chunk):
#   c in 0..6: dest rows 16c+1 .. 16c+16
#   c == 7:    dest rows 113..127 (15 rows)
#   c == 8:    dest row 0 (1 row)
CHUNK_SPECS = [(16 * c + 1, 16) for c in range(7)] + [(113, 15), (0, 1)]
# engine per chunk: 'v' = vector(DVE), 's' = scalar(ACT); ACT is ~2x slower
# so it gets ~1/3 of the rows.  chunk 8 shares blocks with chunk 7 -> same.
CHUNK_ENGINE = ['v', 's', 'v', 's', 'v', 's', 'v', 'v', 'v']


def _row_chunk(d):
    """dest row -> (chunk, rel_row)"""
    if d == 0:
        return 8, 0
    if d <= 112:
        return (d - 1) // 16, (d - 1) % 16
    return 7, d - 113


def _dest_segments(j):
    """For block j (source h rows 4j..4j+3) return segments
    (r_lo, r_hi, chunk, rel_lo): source rows r_lo..r_hi map to dest rows
    rel_lo+cnt-1 .. rel_lo (descending wrt r) inside one chunk."""
    segs = []
    r = 0
    while r < 4:
        d = (H - (4 * j + r)) % H
        c, _ = _row_chunk(d)
        r2 = r
        while r2 + 1 < 4:
            d2 = (H - (4 * j + r2 + 1)) % H
            c2, _ = _row_chunk(d2)
            if c2 != c or d2 != d - (r2 + 1 - r):
                break
            r2 += 1
        d_lo = (H - (4 * j + r2)) % H
        _, rel_lo = _row_chunk(d_lo)
        segs.append((r, r2, c, rel_lo))
        r = r2 + 1
    return segs


def _rev(hi, lo):
    """slice for indices hi..lo inclusive, descending"""
    return slice(hi, None if lo == 0 else lo - 1, -1)


def _build_nc():
    nc = bacc.Bacc(None, target_bir_lowering=False)
    x = nc.dram_tensor("x", [CIN, HW], F32, kind="ExternalInput")
    wT = nc.dram_tensor("wT", [CIN, COUT], F32, kind="ExternalInput")
    y = nc.dram_tensor("y", [COUT, HW], F32, kind="ExternalOutput")

    with tile.TileContext(nc) as tc:
        with (
            tc.tile_pool(name="wp", bufs=1) as wpool,
            tc.tile_pool(name="xp", bufs=1) as xpool,
            tc.tile_pool(name="yp", bufs=1) as ypool,
            tc.tile_pool(name="ps", bufs=6, space="PSUM") as pspool,
            tc.tile_pool(name="pp", bufs=1, space="PSUM") as probepool,
        ):
            w_t = wpool.tile([CIN, COUT], F32)
            nc.sync.dma_start(w_t[:], wT[:])

            scratch = probepool.tile([1, 1], F32, name="probe_ps")

            # j processing order: 0 first, then 31..1 (fills dest rows
            # ascending so output chunks complete evenly)
            j_order = [0] + list(range(NBLK - 1, 0, -1))

            # x chunks: k covers blocks 4k..4k+3 (cols 2048k..2048k+2047)
            xch = {}
            k_order = []
            for j in j_order:
                if j // 4 not in k_order:
                    k_order.append(j // 4)
            for k in k_order:
                t = xpool.tile([CIN, 4, BLK], F32, tag=f"x{k}", name=f"xch{k}")
                nc.sync.dma_start(
                    t[:],
                    x[:, 4 * BLK * k: 4 * BLK * (k + 1)].rearrange(
                        "p (r n) -> p r n", n=BLK
                    ),
                )
                xch[k] = t

            ych = {}
            for c, (_, nrows) in enumerate(CHUNK_SPECS):
                ych[c] = ypool.tile(
                    [COUT, nrows, W], F32, tag=f"y{c}", name=f"ych{c}"
                )

            rows_written = [0] * len(CHUNK_SPECS)
            probed = set()
            for j in j_order:
                k = j // 4
                if k not in probed:
                    # 1-col matmul, both operands from the chunk: absorbs the
                    # chunk-DMA wait on PE so real matmuls don't need it
                    nc.tensor.matmul(
                        scratch[0:1, 0:1],
                        xch[k][:, 0, 0:1],
                        xch[k][:, 0, 0:1],
                        start=True,
                        stop=True,
                    )
                    probed.add(k)
                ps = pspool.tile([COUT, BLK], F32, tag="ps", name=f"ps{j}")
                nc.tensor.matmul(
                    ps[:],
                    w_t[:],
                    xch[k][:, j % 4, :],
                    start=True,
                    stop=True,
                )
                psv = ps[:].rearrange("p (r w) -> p r w", w=W)
                segs = _dest_segments(j)
                eng = CHUNK_ENGINE[segs[0][2]]
                for (r_lo, r_hi, c, rel_lo) in segs:
                    cnt = r_hi - r_lo + 1
                    dst = ych[c]
                    # bulk: dest cols 1..127 <- src cols 127..1 (reversed)
                    bulk_src = psv[:, _rev(r_hi, r_lo), _rev(W - 1, 1)]
                    bulk_dst = dst[:, rel_lo:rel_lo + cnt, 1:W]
                    # w0 column: dest col 0 <- src col 0
                    col_src = psv[:, _rev(r_hi, r_lo), 0:1]
                    col_dst = dst[:, rel_lo:rel_lo + cnt, 0:1]
                    # both copies of one PSUM tile on the SAME engine so the
                    # slot's next matmul needs only one sync wait
                    if eng == 's':
                        nc.scalar.copy(bulk_dst, bulk_src)
                        nc.scalar.copy(col_dst, col_src)
                    else:
                        nc.vector.tensor_copy(bulk_dst, bulk_src)
                        nc.vector.tensor_copy(col_dst, col_src)
                    rows_written[c] += cnt
                for (_, _, c, _) in segs:
                    d0, nrows = CHUNK_SPECS[c]
                    if rows_written[c] == nrows:
                        nc.sync.dma_start(
                            y[:, d0 * W: (d0 + nrows) * W].rearrange(
                                "p (r w) -> p r w", w=W
                            ),
                            ych[c][:],
                        )
                        rows_written[c] = -1  # done
    nc.compile()
    return nc


_NC_CACHE = {}


def _get_nc():
    if "nc" not in _NC_CACHE:
        _NC_CACHE["nc"] = _build_nc()
    return _NC_CACHE["nc"]


def kernel(x, alphas, betas=None, **_unused):
    x = np.ascontiguousarray(x, dtype=np.float32)
    wT = np.ascontiguousarray(alphas.T, dtype=np.float32)
    nc = _get_nc()
    in_maps = [
        {"x": np.ascontiguousarray(x[c].reshape(CIN, HW)), "wT": wT}
        for c in range(N_CORES)
    ]
    res = run_bass_kernel_spmd(nc, in_maps, core_ids=list(range(N_CORES)))
    out = np.stack(
        [res.results[c]["y"].reshape(COUT, H, W) for c in range(N_CORES)]
    )
    return out.astype(np.float32)



# revision 2
# speedup vs baseline: 1.4215x; 1.4215x over previous
"""Trainium2 Bass kernel for nn_KKLayer (spectral channel-mix layer).

Math identity: the reference computes
    y = Re(IFFT2((A + iB) . conj(FFT2(x))))            (channel mix in freq domain)
Since channel mixing commutes with the spatial FFT and, for real x,
IFFT2(conj(FFT2(x))) is x spatially "negated" (h -> (-h) mod H, w -> (-w) mod W),
the whole layer collapses to
    y[b,o,h,w] = sum_i A[o,i] * x[b,i,(H-h)%H,(W-w)%W]
(betas drop out of the real part entirely).

Kernel: data-parallel over batch (8 batches -> 8 cores). Per core:
  - load alphas^T (stationary matmul weights, bf16) + x[b] (bf16) into SBUF
  - 32 bf16 matmuls [K=128,M=128,N=512] -> PSUM fp32
  - PSUM->SBUF copies apply the (h,w) flip via negative-stride APs and
    downconvert fp32 -> bf16
  - contiguous bf16 DMA-out chunks; host upcasts to fp32

bf16 everywhere halves HBM traffic (8.4MB/core vs 16.8MB) and runs the PE
at 1 cycle/row instead of fp32's 4 (rel err ~1e-3 << 2e-2 gate).

Single-wait discipline: TRN2 instructions carry at most ONE semaphore wait.
 - a 1-col "probe" matmul per x-chunk (both operands from the chunk) absorbs
   the chunk-DMA wait on PE; real matmuls then only wait on PSUM-slot reuse
 - all copies feeding one output chunk run on one engine, so each output DMA
   and each PSUM-slot reuse waits on a single engine
"""

import ml_dtypes
import numpy as np

import concourse.bass as bass
import concourse.bacc as bacc
import concourse.mybir as mybir
from concourse import tile
from concourse.bass_utils import run_bass_kernel_spmd

B, CIN, COUT, H, W = 8, 128, 128, 128, 128
HW = H * W          # 16384
BLK = 512           # matmul free dim (one PSUM bank of fp32)
NBLK = HW // BLK    # 32 blocks; block j covers h rows 4j..4j+3
N_CORES = 8

F32 = mybir.dt.float32
BF16 = mybir.dt.bfloat16
NP_BF16 = ml_dtypes.bfloat16

# output chunks (offset by 1 row so no 4-row block straddles a chunk):
#   c in 0..6: dest rows 16c+1 .. 16c+16
#   c == 7:    dest rows 113..127 (15 rows)
#   c == 8:    dest row 0 (1 row)
CHUNK_SPECS = [(16 * c + 1, 16) for c in range(7)] + [(113, 15), (0, 1)]
# engine per chunk: 'v' = vector(DVE), 's' = scalar(ACT); ACT is ~2x slower
# so it gets ~1/3 of the rows.  chunk 8 shares blocks with chunk 7 -> same.
CHUNK_ENGINE = ['v', 's', 'v', 's', 'v', 's', 'v', 'v', 'v']


def _row_chunk(d):
    """dest row -> (chunk, rel_row)"""
    if d == 0:
        return 8, 0
    if d <= 112:
        return (d - 1) // 16, (d - 1) % 16
    return 7, d - 113


def _dest_segments(j):
    """For block j (source h rows 4j..4j+3) return segments
    (r_lo, r_hi, chunk, rel_lo): source rows r_lo..r_hi map to dest rows
    rel_lo+cnt-1 .. rel_lo (descending wrt r) inside one chunk."""
    segs = []
    r = 0
    while r < 4:
        d = (H - (4 * j + r)) % H
        c, _ = _row_chunk(d)
        r2 = r
        while r2 + 1 < 4:
            d2 = (H - (4 * j + r2 + 1)) % H
            c2, _ = _row_chunk(d2)
            if c2 != c or d2 != d - (r2 + 1 - r):
                break
            r2 += 1
        d_lo = (H - (4 * j + r2)) % H
        _, rel_lo = _row_chunk(d_lo)
        segs.append((r, r2, c, rel_lo))
        r = r2 + 1
    return segs


def _rev(hi, lo):
    """slice for indices hi..lo inclusive, descending"""
    return slice(hi, None if lo == 0 else lo - 1, -1)


def _build_nc():
    nc = bacc.Bacc(None, target_bir_lowering=False)
    x = nc.dram_tensor("x", [CIN, HW], BF16, kind="ExternalInput")
    wT = nc.dram_tensor("wT", [CIN, COUT], BF16, kind="ExternalInput")
    y = nc.dram_tensor("y", [COUT, HW], BF16, kind="ExternalOutput")

    with tile.TileContext(nc) as tc:
        with (
            tc.tile_pool(name="wp", bufs=1) as wpool,
            tc.tile_pool(name="xp", bufs=1) as xpool,
            tc.tile_pool(name="yp", bufs=1) as ypool,
            tc.tile_pool(name="ps", bufs=6, space="PSUM") as pspool,
            tc.tile_pool(name="pp", bufs=1, space="PSUM") as probepool,
        ):
            w_t = wpool.tile([CIN, COUT], BF16)
            nc.sync.dma_start(w_t[:], wT[:])

            scratch = probepool.tile([1, 1], F32, name="probe_ps")

            # j processing order: 0 first, then 31..1 (fills dest rows
            # ascending so output chunks complete evenly)
            j_order = [0] + list(range(NBLK - 1, 0, -1))

            # x chunks: k covers blocks 4k..4k+3 (cols 2048k..2048k+2047)
            xch = {}
            k_order = []
            for j in j_order:
                if j // 4 not in k_order:
                    k_order.append(j // 4)
            for k in k_order:
                t = xpool.tile([CIN, 4, BLK], BF16, tag=f"x{k}", name=f"xch{k}")
                nc.sync.dma_start(
                    t[:],
                    x[:, 4 * BLK * k: 4 * BLK * (k + 1)].rearrange(
                        "p (r n) -> p r n", n=BLK
                    ),
                )
                xch[k] = t

            ych = {}
            for c, (_, nrows) in enumerate(CHUNK_SPECS):
                ych[c] = ypool.tile(
                    [COUT, nrows, W], BF16, tag=f"y{c}", name=f"ych{c}"
                )

            rows_written = [0] * len(CHUNK_SPECS)
            probed = set()
            for j in j_order:
                k = j // 4
                if k not in probed:
                    # 1-col matmul, both operands from the chunk: absorbs the
                    # chunk-DMA wait on PE so real matmuls don't need it
                    nc.tensor.matmul(
                        scratch[0:1, 0:1],
                        xch[k][:, 0, 0:1],
                        xch[k][:, 0, 0:1],
                        start=True,
                        stop=True,
                    )
                    probed.add(k)
                ps = pspool.tile([COUT, BLK], F32, tag="ps", name=f"ps{j}")
                nc.tensor.matmul(
                    ps[:],
                    w_t[:],
                    xch[k][:, j % 4, :],
                    start=True,
                    stop=True,
                )
                psv = ps[:].rearrange("p (r w) -> p r w", w=W)
                segs = _dest_segments(j)
                eng = CHUNK_ENGINE[segs[0][2]]
                for (r_lo, r_hi, c, rel_lo) in segs:
                    cnt = r_hi - r_lo + 1
                    dst = ych[c]
                    # bulk: dest cols 1..127 <- src cols 127..1 (reversed)
                    bulk_src = psv[:, _rev(r_hi, r_lo), _rev(W - 1, 1)]
                    bulk_dst = dst[:, rel_lo:rel_lo + cnt, 1:W]
                    # w0 column: dest col 0 <- src col 0
                    col_src = psv[:, _rev(r_hi, r_lo), 0:1]
                    col_dst = dst[:, rel_lo:rel_lo + cnt, 0:1]
                    # both copies of one PSUM tile on the SAME engine so the
                    # slot's next matmul needs only one sync wait
                    if eng == 's':
                        nc.scalar.copy(bulk_dst, bulk_src)
                        nc.scalar.copy(col_dst, col_src)
                    else:
                        nc.vector.tensor_copy(bulk_dst, bulk_src)
                        nc.vector.tensor_copy(col_dst, col_src)
                    rows_written[c] += cnt
                for (_, _, c, _) in segs:
                    d0, nrows = CHUNK_SPECS[c]
                    if rows_written[c] == nrows:
                        nc.sync.dma_start(
                            y[:, d0 * W: (d0 + nrows) * W].rearrange(
                                "p (r w) -> p r w", w=W
                            ),
                            ych[c][:],
                        )
                        rows_written[c] = -1  # done
    nc.compile()
    return nc


_NC_CACHE = {}


def _get_nc():
    if "nc" not in _NC_CACHE:
        _NC_CACHE["nc"] = _build_nc()
    return _NC_CACHE["nc"]


def make_in_maps(x, alphas):
    """Per-core input maps (bf16)."""
    x16 = np.ascontiguousarray(x, dtype=np.float32).astype(NP_BF16)
    wT = np.ascontiguousarray(
        np.asarray(alphas, dtype=np.float32).T
    ).astype(NP_BF16)
    return [
        {"x": np.ascontiguousarray(x16[c].reshape(CIN, HW)), "wT": wT}
        for c in range(N_CORES)
    ]


def kernel(x, alphas, betas=None, **_unused):
    nc = _get_nc()
    in_maps = make_in_maps(x, alphas)
    res = run_bass_kernel_spmd(nc, in_maps, core_ids=list(range(N_CORES)))
    out = np.stack(
        [
            res.results[c]["y"].astype(np.float32).reshape(COUT, H, W)
            for c in range(N_CORES)
        ]
    )
    return out
